# revision 8
# baseline (speedup 1.0000x reference)
"""Causal attention kernel for Trainium2 (Bass/Tile), SPMD over 8 NeuronCores.

Problem: B=16, N=2048, D=256 fp32 causal attention with padding mask.
Sharding: batch dim across 8 cores (2 batches per core); attention is
batch-independent so no collectives are needed.

The PE processes one moving-operand row per cycle at 2.4GHz regardless of
dtype (measured: bf16 = fp16 = fp8 = fp8-DoubleRow = fp32r all ~0.42ns/row),
so the fastest correct configuration is plain fp16 everywhere:

  S^T = K @ Q^T   per 128-k-row chunk, 2 fp16 matmuls (d-chunks of 128),
                  diagonal chunks trimmed to the unmasked columns
  P^T = exp(scale * S^T)  on ScalarE, fp16 out; chunk PAIRS share one
                  [128,1024] PSUM tile so one activation covers two chunks
                  (halves ScalarE per-instruction overhead); pairs touching
                  the diagonal use two trimmed activations instead
  [O | rowsum] = P @ [V | 1]  fp16 PV with a ones-column so the softmax
                  denominators fall out of the same matmuls
  O = O * (1/rowsum)  on DVE, streamed out per q-block

Scheduling: all K/Q input DMAs ride the SP HWDGE queue (leading chunks
front-loaded so the first matmul starts ~1us after the preamble), V and
outputs ride the Pool SWDGE queue, and the Activation queue is kept
DMA-free — DMA issue on the act queue was measured to delay activations
(and thus the PE) by multiple us.
"""

import numpy as np

import concourse.bass as bass
from concourse import bacc
import concourse.mybir as mybir
from concourse import tile
from concourse.bass_utils import run_bass_kernel_spmd

F32 = mybir.dt.float32
F16 = mybir.dt.float16
I32 = mybir.dt.int32

N_CORES = 8
B_FULL, N_SEQ, D_MODEL = 16, 2048, 256
B_LOCAL = B_FULL // N_CORES

NEG = -1e30
P = 128


def build_attention_nc(B=B_LOCAL, N=N_SEQ, D=D_MODEL, QBS=512, pad=False):
    nc = bacc.Bacc(num_swdge_queues=2)
    NT = N // P            # 128-row tiles along sequence
    DC = D // P            # 128-wide d chunks
    TB = QBS // P          # q tiles per q block
    NB = N // QBS          # q blocks
    DP = D + 8             # PV moving width (ones col at D, zero pad to 16B)
    scale = 1.0 / float(np.sqrt(D))

    qt_d = nc.declare_dram_parameter("qt", [B, P, DC, N], F16, isOutput=False)
    kt_d = nc.declare_dram_parameter("kt", [B, P, DC, N], F16, isOutput=False)
    v_d = nc.declare_dram_parameter("v", [B, N, D], F16, isOutput=False)
    if pad:
        pm_d = nc.declare_dram_parameter("pm", [B, N], I32, isOutput=False)
    o_d = nc.declare_dram_parameter("o", [B, N, D], F32, isOutput=True)

    with tile.TileContext(nc) as tc:
        with (
            tc.tile_pool(name="consts", bufs=1) as consts,
            tc.tile_pool(name="big", bufs=2) as big,
            tc.tile_pool(name="ptp", bufs=4) as ptp,
            tc.tile_pool(name="smallp", bufs=4) as smallp,
            tc.tile_pool(name="ps_sp", bufs=2, space="PSUM") as ps_sp,
            tc.tile_pool(name="ps_op", bufs=TB, space="PSUM") as ps_op,
        ):
            # Additive causal mask for the diagonal 128x128 chunk of S^T:
            # [k_local, q_local] kept iff k <= q.
            dmask = consts.tile([P, P], F32)
            nc.gpsimd.memset(dmask, 0.0)
            nc.gpsimd.affine_select(
                out=dmask,
                in_=dmask,
                compare_op=mybir.AluOpType.is_ge,
                fill=NEG,
                base=0,
                pattern=[[1, P]],
                channel_multiplier=-1,
            )

            for b in range(B):
                # head tiles: first q-block operands land in their own tiles
                # (single DMA each) so the first matmuls' semaphore waits are
                # precise instead of dragging behind the whole input queue
                kt_h = big.tile([P, DC, QBS], F16, tag="kt_h")
                qt_h = big.tile([P, DC, QBS], F16, tag="qt_h")
                kt = big.tile([P, DC, N], F16, tag="kt")
                qt = big.tile([P, DC, N], F16, tag="qt")
                vx = big.tile([P, NT, DP], F16, tag="vx")
                ostg = big.tile([P, NT, D], F32, tag="ostg")

                def kt_sl(dc, c0, c1):
                    if c1 <= QBS // P:
                        return kt_h[:, dc, c0 * P : c1 * P]
                    return kt[:, dc, c0 * P : c1 * P]

                def qt_sl(dc, q0, q1):
                    if q1 <= QBS:
                        return qt_h[:, dc, q0:q1]
                    return qt[:, dc, q0:q1]

                # ones column (softmax denominator) + zero pad
                ones_t = smallp.tile([P, NT, 8], F16, tag="ones")
                nc.gpsimd.memset(ones_t, 0.0)
                nc.gpsimd.memset(ones_t[:, :, 0], 1.0)
                nc.vector.tensor_copy(vx[:, :, D : D + 8], ones_t)

                if pad:
                    pbias = big.tile([P, NT], F32, tag="pbias")
                    pmi = smallp.tile([P, NT], I32, tag="pmi")
                    nc.sync.dma_start(
                        out=pmi, in_=pm_d[b].rearrange("(c p) -> p c", p=P)
                    )
                    pmf = smallp.tile([P, NT], F32, tag="pmf")
                    nc.vector.tensor_copy(pmf, pmi)
                    tmp = smallp.tile([P, NT], F32, tag="tmp")
                    nc.vector.tensor_scalar(
                        out=tmp,
                        in0=pmf,
                        scalar1=1.0,
                        scalar2=None,
                        op0=mybir.AluOpType.min,
                    )
                    nc.vector.tensor_scalar(
                        out=pbias,
                        in0=tmp,
                        scalar1=-1.0,
                        scalar2=-NEG,
                        op0=mybir.AluOpType.add,
                        op1=mybir.AluOpType.mult,
                    )

                # ---- input DMA schedule (K/Q on the SP HWDGE queue, V on
                # the Pool SWDGE queue; first q-block operands front-loaded)
                kt_r, qt_r = kt_d[b], qt_d[b]
                v_r = v_d[b].rearrange("(c p) d -> p c d", p=P)
                nc.sync.dma_start(out=kt_h, in_=kt_r[:, :, 0:QBS])
                nc.sync.dma_start(out=qt_h, in_=qt_r[:, :, 0:QBS])
                G = 4  # V chunks per DMA group
                nc.gpsimd.dma_start(out=vx[:, 0:G, 0:D], in_=v_r[:, 0:G, :])
                for h in range(1, NB):
                    sl = slice(h * QBS, (h + 1) * QBS)
                    nc.sync.dma_start(out=qt[:, :, sl], in_=qt_r[:, :, sl])
                    nc.sync.dma_start(out=kt[:, :, sl], in_=kt_r[:, :, sl])
                    nc.gpsimd.dma_start(
                        out=vx[:, h * G : (h + 1) * G, 0:D],
                        in_=v_r[:, h * G : (h + 1) * G, :],
                    )

                o_r = o_d[b].rearrange("(c p) d -> p c d", p=P)

                # ---- main attention loop over q blocks ----
                # PE emission is software-pipelined: QK(m+1) is emitted
                # before PV(m) so the activation latency at block starts is
                # covered by the next pair's QK matmuls.
                for qb in range(NB):
                    tbase = qb * TB
                    C = tbase + TB  # k chunks this block
                    M = C // 2
                    po = [
                        ps_op.tile([P, DP], F32, tag="po", name=f"po{qb % 2}_{i}")
                        for i in range(TB)
                    ]
                    ss_t = [None] * M
                    pt_t = [None] * M

                    def emit_qk(m, qb=qb, tbase=tbase, ss_t=ss_t):
                        ss = ps_sp.tile([P, 2 * QBS], F32, tag="ss")
                        ss_t[m] = ss
                        for ci, c in enumerate((2 * m, 2 * m + 1)):
                            ls = max(0, (c - tbase) * P)
                            base = ci * QBS
                            for dc in range(DC):
                                nc.tensor.matmul(
                                    ss[:, base + ls : base + QBS],
                                    kt_sl(dc, c, c + 1),
                                    qt_sl(dc, qb * QBS + ls, (qb + 1) * QBS),
                                    start=(dc == 0),
                                    stop=(dc == DC - 1),
                                )
                            if c >= tbase:
                                i = c - tbase
                                sl = slice(base + i * P, base + (i + 1) * P)
                                nc.vector.tensor_add(ss[:, sl], ss[:, sl], dmask)

                    def emit_act(m, tbase=tbase, ss_t=ss_t, pt_t=pt_t):
                        c_pair = (2 * m, 2 * m + 1)
                        ss = ss_t[m]
                        pt = ptp.tile([P, 2 * QBS], F16, tag="pt")
                        pt_t[m] = pt
                        # split the act for pairs on the diagonal (trimmed
                        # inputs) and for the first pair of each block (so the
                        # first PV only waits on a half-width activation)
                        if pad or c_pair[1] > tbase or m == 0:
                            for ci, c in enumerate(c_pair):
                                ls = max(0, (c - tbase) * P)
                                sl = slice(ci * QBS + ls, (ci + 1) * QBS)
                                kw = (
                                    {"bias": pbias[:, c : c + 1]} if pad else {}
                                )
                                nc.scalar.activation(
                                    pt[:, sl],
                                    ss[:, sl],
                                    mybir.ActivationFunctionType.Exp,
                                    scale=scale,
                                    **kw,
                                )
                        else:
                            nc.scalar.activation(
                                pt[:, 0 : 2 * QBS],
                                ss[:, 0 : 2 * QBS],
                                mybir.ActivationFunctionType.Exp,
                                scale=scale,
                            )

                    def emit_pv(m, qb=qb, tbase=tbase, pt_t=pt_t, po=po):
                        pt = pt_t[m]
                        for ci, c in enumerate((2 * m, 2 * m + 1)):
                            for ti in range(max(0, c - tbase), TB):
                                t = tbase + ti
                                nc.tensor.matmul(
                                    po[ti],
                                    pt[:, ci * QBS + ti * P : ci * QBS + (ti + 1) * P],
                                    vx[:, c, 0:DP],
                                    start=(c == 0),
                                    stop=(c == t),
                                )
                                if c == t:
                                    # drain this tile now: scale by the
                                    # reciprocal rowsum and stage for DMA
                                    rec = smallp.tile([P, 1], F32, tag="rec")
                                    nc.vector.reciprocal(
                                        rec, po[ti][:, D : D + 1]
                                    )
                                    nc.vector.tensor_scalar_mul(
                                        ostg[:, t, :], po[ti][:, 0:D], rec
                                    )
                                    if qb == NB - 1:
                                        # tail: per-tile, on the (now idle)
                                        # SP queue, as soon as it's scaled
                                        nc.sync.dma_start(
                                            out=o_r[:, t : t + 1, :],
                                            in_=ostg[:, t : t + 1, :],
                                        )

                    emit_qk(0)
                    emit_act(0)
                    if M > 1:
                        emit_qk(1)
                        emit_act(1)
                    for m in range(M):
                        emit_pv(m)
                        if m + 2 < M:
                            emit_qk(m + 2)
                            emit_act(m + 2)
                    if qb < NB - 1:
                        nc.gpsimd.dma_start(
                            out=o_r[:, tbase : tbase + TB, :],
                            in_=ostg[:, tbase : tbase + TB, :],
                        )

    nc.finalize()
    return nc


_NC_CACHE = {}


def _get_nc(pad=False):
    key = (B_LOCAL, N_SEQ, D_MODEL, pad)
    if key not in _NC_CACHE:
        _NC_CACHE[key] = build_attention_nc(pad=pad)
    return _NC_CACHE[key]


def _t16(x):
    """x [b, N, D] fp32 -> transposed fp16 [b, 128, DC, N] with d=dc*128+p."""
    b, n, d = x.shape
    xt = np.ascontiguousarray(x.transpose(0, 2, 1)).astype(np.float16)
    return np.ascontiguousarray(
        xt.reshape(b, d // P, P, n).transpose(0, 2, 1, 3)
    )


def _prep(Q, K, V, padding_mask):
    Q = np.asarray(Q, dtype=np.float32)
    K = np.asarray(K, dtype=np.float32)
    V16 = np.ascontiguousarray(np.asarray(V, dtype=np.float32).astype(np.float16))
    pm = np.ascontiguousarray(np.asarray(padding_mask), dtype=np.int32)
    pad = not bool(np.all(pm != 0))

    qt = _t16(Q)
    kt = _t16(K)

    nc = _get_nc(pad=pad)
    in_maps = []
    for c in range(N_CORES):
        s = slice(c * B_LOCAL, (c + 1) * B_LOCAL)
        m = {"qt": qt[s], "kt": kt[s], "v": V16[s]}
        if pad:
            m["pm"] = pm[s]
        in_maps.append(m)
    return nc, in_maps


def kernel(Q, K, V, padding_mask):
    nc, in_maps = _prep(Q, K, V, padding_mask)
    res = run_bass_kernel_spmd(nc, in_maps, list(range(N_CORES)))
    out = np.concatenate([res.results[c]["o"] for c in range(N_CORES)], axis=0)
    return out.astype(np.float32)


# revision 12
# speedup vs baseline: 1.1740x; 1.1740x over previous
"""Causal attention kernel for Trainium2 (Bass/Tile), SPMD over 8 NeuronCores.

Problem: B=16, N=2048, D=256 fp32 causal attention with padding mask.
Sharding: batch dim across 8 cores (2 batches per core); attention is
batch-independent so no collectives are needed.

The PE processes one moving-operand row per cycle at 2.4GHz regardless of
dtype (measured: bf16 = fp16 = fp8 = fp8-DoubleRow = fp32r all ~0.42ns/row),
so the fastest correct configuration is plain fp16 everywhere:

  S^T = K @ Q^T   per 128-k-row chunk, 2 fp16 matmuls (d-chunks of 128),
                  diagonal chunks trimmed to the unmasked columns
  P^T = exp(scale * S^T)  on ScalarE, fp16 out; chunk PAIRS share one
                  [128,1024] PSUM tile so one activation covers two chunks
                  (halves ScalarE per-instruction overhead); pairs touching
                  the diagonal use two trimmed activations instead
  [O | rowsum] = P @ [V | 1]  fp16 PV with a ones-column so the softmax
                  denominators fall out of the same matmuls
  O = O * (1/rowsum)  on DVE, streamed out per q-block

Scheduling: all K/Q input DMAs ride the SP HWDGE queue (leading chunks
front-loaded so the first matmul starts ~1us after the preamble), V and
outputs ride the Pool SWDGE queue, and the Activation queue is kept
DMA-free — DMA issue on the act queue was measured to delay activations
(and thus the PE) by multiple us.
"""

import numpy as np

import concourse.bass as bass
from concourse import bacc
import concourse.mybir as mybir
from concourse import tile
from concourse.bass_utils import run_bass_kernel_spmd

F32 = mybir.dt.float32
F16 = mybir.dt.float16
I32 = mybir.dt.int32

N_CORES = 8
B_FULL, N_SEQ, D_MODEL = 16, 2048, 256
B_LOCAL = B_FULL // N_CORES

NEG = -1e30
P = 128


def build_attention_nc(B=B_LOCAL, N=N_SEQ, D=D_MODEL, QBS=512, pad=False):
    nc = bacc.Bacc(num_swdge_queues=2)
    NT = N // P            # 128-row tiles along sequence
    DC = D // P            # 128-wide d chunks
    TB = QBS // P          # q tiles per q block
    NB = N // QBS          # q blocks
    DP = D + 8             # PV moving width (ones col at D, zero pad to 16B)
    scale = 1.0 / float(np.sqrt(D))

    qt_d = nc.declare_dram_parameter("qt", [B, P, DC, N], F16, isOutput=False)
    kt_d = nc.declare_dram_parameter("kt", [B, P, DC, N], F16, isOutput=False)
    v_d = nc.declare_dram_parameter("v", [B, N, D], F16, isOutput=False)
    if pad:
        pm_d = nc.declare_dram_parameter("pm", [B, N], I32, isOutput=False)
    o_d = nc.declare_dram_parameter("o", [B, N, D], F32, isOutput=True)

    with tile.TileContext(nc) as tc:
        with (
            tc.tile_pool(name="consts", bufs=1) as consts,
            tc.tile_pool(name="big", bufs=2) as big,
            tc.tile_pool(name="ptp", bufs=4) as ptp,
            tc.tile_pool(name="smallp", bufs=4) as smallp,
            tc.tile_pool(name="ps_sp", bufs=2, space="PSUM") as ps_sp,
            tc.tile_pool(name="ps_op", bufs=TB, space="PSUM") as ps_op,
        ):
            # Additive causal mask for the diagonal 128x128 chunk of S^T:
            # [k_local, q_local] kept iff k <= q.
            dmask = consts.tile([P, P], F32)
            nc.gpsimd.memset(dmask, 0.0)
            nc.gpsimd.affine_select(
                out=dmask,
                in_=dmask,
                compare_op=mybir.AluOpType.is_ge,
                fill=NEG,
                base=0,
                pattern=[[1, P]],
                channel_multiplier=-1,
            )
            # prime the ScalarE Exp table during the preamble — the lazy
            # ACT_TABLE_LOAD otherwise costs ~1.5us right before the first
            # real activation
            warm = consts.tile([P, 1], F16)
            nc.scalar.activation(
                warm, dmask[:, 0:1], mybir.ActivationFunctionType.Exp, scale=1.0
            )

            for b in range(B):
                # head tiles: first q-block operands land in their own tiles
                # (single DMA each) so the first matmuls' semaphore waits are
                # precise instead of dragging behind the whole input queue
                kt_h = big.tile([P, DC, QBS], F16, tag="kt_h")
                qt_h = big.tile([P, DC, QBS], F16, tag="qt_h")
                kt = big.tile([P, DC, N], F16, tag="kt")
                qt = big.tile([P, DC, N], F16, tag="qt")
                vx = big.tile([P, NT, DP], F16, tag="vx")
                ostg = big.tile([P, NT, D], F32, tag="ostg")

                def kt_sl(dc, c0, c1):
                    if c1 <= QBS // P:
                        return kt_h[:, dc, c0 * P : c1 * P]
                    return kt[:, dc, c0 * P : c1 * P]

                def qt_sl(dc, q0, q1):
                    if q1 <= QBS:
                        return qt_h[:, dc, q0:q1]
                    return qt[:, dc, q0:q1]

                # ones column (softmax denominator) + zero pad
                ones_t = smallp.tile([P, NT, 8], F16, tag="ones")
                nc.gpsimd.memset(ones_t, 0.0)
                nc.gpsimd.memset(ones_t[:, :, 0], 1.0)
                nc.vector.tensor_copy(vx[:, :, D : D + 8], ones_t)

                if pad:
                    pbias = big.tile([P, NT], F32, tag="pbias")
                    pmi = smallp.tile([P, NT], I32, tag="pmi")
                    nc.sync.dma_start(
                        out=pmi, in_=pm_d[b].rearrange("(c p) -> p c", p=P)
                    )
                    pmf = smallp.tile([P, NT], F32, tag="pmf")
                    nc.vector.tensor_copy(pmf, pmi)
                    tmp = smallp.tile([P, NT], F32, tag="tmp")
                    nc.vector.tensor_scalar(
                        out=tmp,
                        in0=pmf,
                        scalar1=1.0,
                        scalar2=None,
                        op0=mybir.AluOpType.min,
                    )
                    nc.vector.tensor_scalar(
                        out=pbias,
                        in0=tmp,
                        scalar1=-1.0,
                        scalar2=-NEG,
                        op0=mybir.AluOpType.add,
                        op1=mybir.AluOpType.mult,
                    )

                # ---- input DMA schedule (K/Q on the SP HWDGE queue, V on
                # the Pool SWDGE queue; first q-block operands front-loaded)
                kt_r, qt_r = kt_d[b], qt_d[b]
                v_r = v_d[b].rearrange("(c p) d -> p c d", p=P)
                nc.sync.dma_start(out=kt_h, in_=kt_r[:, :, 0:QBS])
                nc.sync.dma_start(out=qt_h, in_=qt_r[:, :, 0:QBS])
                # Only V group 0 up front: the PE's pool-queue semaphore
                # waits are coalesced, so any V DMA issued before a q-block
                # gates that block's first matmul. Groups 1..3 are issued one
                # q-block ahead inside the main loop.
                G = 4  # V chunks per DMA group
                nc.gpsimd.dma_start(out=vx[:, 0:G, 0:D], in_=v_r[:, 0:G, :])
                for h in range(1, NB):
                    sl = slice(h * QBS, (h + 1) * QBS)
                    nc.sync.dma_start(out=qt[:, :, sl], in_=qt_r[:, :, sl])
                    nc.sync.dma_start(out=kt[:, :, sl], in_=kt_r[:, :, sl])

                o_r = o_d[b].rearrange("(c p) d -> p c d", p=P)

                # ---- main attention loop over q blocks ----
                # PE emission is software-pipelined: QK(m+1) is emitted
                # before PV(m) so the activation latency at block starts is
                # covered by the next pair's QK matmuls.
                for qb in range(NB):
                    tbase = qb * TB
                    C = tbase + TB  # k chunks this block
                    M = C // 2
                    po = [
                        ps_op.tile([P, DP], F32, tag="po", name=f"po{qb % 2}_{i}")
                        for i in range(TB)
                    ]
                    ss_t = [None] * M
                    pt_t = [None] * M

                    def emit_qk(m, qb=qb, tbase=tbase, ss_t=ss_t):
                        ss = ps_sp.tile([P, 2 * QBS], F32, tag="ss")
                        ss_t[m] = ss
                        for ci, c in enumerate((2 * m, 2 * m + 1)):
                            ls = max(0, (c - tbase) * P)
                            base = ci * QBS
                            for dc in range(DC):
                                nc.tensor.matmul(
                                    ss[:, base + ls : base + QBS],
                                    kt_sl(dc, c, c + 1),
                                    qt_sl(dc, qb * QBS + ls, (qb + 1) * QBS),
                                    start=(dc == 0),
                                    stop=(dc == DC - 1),
                                )
                            if c >= tbase:
                                i = c - tbase
                                sl = slice(base + i * P, base + (i + 1) * P)
                                nc.vector.tensor_add(ss[:, sl], ss[:, sl], dmask)

                    def emit_act(m, tbase=tbase, ss_t=ss_t, pt_t=pt_t):
                        c_pair = (2 * m, 2 * m + 1)
                        ss = ss_t[m]
                        pt = ptp.tile([P, 2 * QBS], F16, tag="pt")
                        pt_t[m] = pt
                        # split the act for pairs on the diagonal (trimmed
                        # inputs) and for the first pair of each block (so the
                        # first PV only waits on a half-width activation)
                        if pad or c_pair[1] > tbase or m == 0:
                            for ci, c in enumerate(c_pair):
                                ls = max(0, (c - tbase) * P)
                                sl = slice(ci * QBS + ls, (ci + 1) * QBS)
                                kw = (
                                    {"bias": pbias[:, c : c + 1]} if pad else {}
                                )
                                nc.scalar.activation(
                                    pt[:, sl],
                                    ss[:, sl],
                                    mybir.ActivationFunctionType.Exp,
                                    scale=scale,
                                    **kw,
                                )
                        else:
                            nc.scalar.activation(
                                pt[:, 0 : 2 * QBS],
                                ss[:, 0 : 2 * QBS],
                                mybir.ActivationFunctionType.Exp,
                                scale=scale,
                            )

                    def emit_pv(m, qb=qb, tbase=tbase, pt_t=pt_t, po=po):
                        pt = pt_t[m]
                        for ci, c in enumerate((2 * m, 2 * m + 1)):
                            for ti in range(max(0, c - tbase), TB):
                                t = tbase + ti
                                nc.tensor.matmul(
                                    po[ti],
                                    pt[:, ci * QBS + ti * P : ci * QBS + (ti + 1) * P],
                                    vx[:, c, 0:DP],
                                    start=(c == 0),
                                    stop=(c == t),
                                )
                                if c == t:
                                    # drain this tile now: scale by the
                                    # reciprocal rowsum and stage for DMA
                                    rec = smallp.tile([P, 1], F32, tag="rec")
                                    nc.vector.reciprocal(
                                        rec, po[ti][:, D : D + 1]
                                    )
                                    nc.vector.tensor_scalar_mul(
                                        ostg[:, t, :], po[ti][:, 0:D], rec
                                    )
                                    if qb == NB - 1:
                                        # tail: per-tile, on the (now idle)
                                        # SP queue, as soon as it's scaled
                                        nc.sync.dma_start(
                                            out=o_r[:, t : t + 1, :],
                                            in_=ostg[:, t : t + 1, :],
                                        )

                    emit_qk(0)
                    emit_act(0)
                    if M > 1:
                        emit_qk(1)
                        emit_act(1)
                    for m in range(M):
                        emit_pv(m)
                        if m + 2 < M:
                            emit_qk(m + 2)
                            emit_act(m + 2)
                    if qb < NB - 1:
                        h = qb + 1
                        nc.gpsimd.dma_start(
                            out=vx[:, h * G : (h + 1) * G, 0:D],
                            in_=v_r[:, h * G : (h + 1) * G, :],
                        )
                        nc.gpsimd.dma_start(
                            out=o_r[:, tbase : tbase + TB, :],
                            in_=ostg[:, tbase : tbase + TB, :],
                        )

    nc.finalize()
    return nc


_NC_CACHE = {}


def _get_nc(pad=False):
    key = (B_LOCAL, N_SEQ, D_MODEL, pad)
    if key not in _NC_CACHE:
        _NC_CACHE[key] = build_attention_nc(pad=pad)
    return _NC_CACHE[key]


def _t16(x):
    """x [b, N, D] fp32 -> transposed fp16 [b, 128, DC, N] with d=dc*128+p."""
    b, n, d = x.shape
    xt = np.ascontiguousarray(x.transpose(0, 2, 1)).astype(np.float16)
    return np.ascontiguousarray(
        xt.reshape(b, d // P, P, n).transpose(0, 2, 1, 3)
    )


def _prep(Q, K, V, padding_mask):
    Q = np.asarray(Q, dtype=np.float32)
    K = np.asarray(K, dtype=np.float32)
    V16 = np.ascontiguousarray(np.asarray(V, dtype=np.float32).astype(np.float16))
    pm = np.ascontiguousarray(np.asarray(padding_mask), dtype=np.int32)
    pad = not bool(np.all(pm != 0))

    qt = _t16(Q)
    kt = _t16(K)

    nc = _get_nc(pad=pad)
    in_maps = []
    for c in range(N_CORES):
        s = slice(c * B_LOCAL, (c + 1) * B_LOCAL)
        m = {"qt": qt[s], "kt": kt[s], "v": V16[s]}
        if pad:
            m["pm"] = pm[s]
        in_maps.append(m)
    return nc, in_maps


def kernel(Q, K, V, padding_mask):
    nc, in_maps = _prep(Q, K, V, padding_mask)
    res = run_bass_kernel_spmd(nc, in_maps, list(range(N_CORES)))
    out = np.concatenate([res.results[c]["o"] for c in range(N_CORES)], axis=0)
    return out.astype(np.float32)
